# revision 7
# baseline (speedup 1.0000x reference)
"""GCN generator kernel for Trainium2 (Bass/Tile), data-parallel over batch.

Full inputs: x [32,128,128] f32, W [128,128] f32 -> adj_output [32,128,128] f32.
Shards batch over 8 NeuronCores (4 per core); the whole 127-step recurrence
runs out of SBUF. See reference semantics notes inline.

Math restructuring (validated against the jax reference to ~1e-8 in fp64/fp32):
  - at step i the normalized adj is blockdiag(M_{i+1}, I): rows/cols > i are
    identity and stay identity under renormalization, so all per-step work is
    restricted to the top-left (i+1)x(i+1) block (padded to even size `me`).
  - prob = x[:, :i] . x[:, i] is written raw into row/col i of both adj and
    adj_output (entries j < i only; diagonals stay); deg over the block equals
    full-row sums; the row-i write goes via DMA (compute engines cannot write
    at an arbitrary partition offset) and deg is computed from the stale block
    plus a rank-1 matmul fixup so the DMA stays off the critical path.
  - matmuls run in float32r (~1.2e-4 relative error, up to 4x faster than
    fp32 on the PE); the normalization outer product stays fp32.
"""
import numpy as np

B, N, D = 32, 128, 128
B_LOC = 4  # batches per core
N_CORES = 8

_nc_cache = {}


def _build_nc():
    import sys

    if "/opt/trn_rl_repo" not in sys.path:
        sys.path.insert(0, "/opt/trn_rl_repo")
    import concourse.bacc as bacc
    import concourse.tile as tile
    from concourse import mybir
    from concourse.masks import make_identity

    f32 = mybir.dt.float32
    f32r = mybir.dt.float32r
    AFT = mybir.ActivationFunctionType

    nc = bacc.Bacc(name="gcn_generator")
    x_in = nc.dram_tensor("x", [B_LOC, N, D], f32, kind="ExternalInput")
    w_in = nc.dram_tensor("W", [D, D], f32, kind="ExternalInput")
    out_d = nc.dram_tensor("out", [B_LOC, N, N], f32, kind="ExternalOutput")

    with tile.TileContext(nc) as tc:
        with (
            tc.tile_pool(name="state", bufs=1) as state,
            tc.tile_pool(name="work", bufs=2) as work,
            tc.tile_pool(name="ps_a", bufs=1, space="PSUM") as ps_a,
            tc.tile_pool(name="ps_b", bufs=2, space="PSUM") as ps_b,
            tc.tile_pool(name="ps_c", bufs=1, space="PSUM") as ps_c,
        ):
            # ---------------- persistent state ----------------
            # adj (cols 0:512) and adj_output (cols 512:1024), 4 batch blocks
            # of 128 cols each inside each half; f32r throughout.
            big = state.tile([128, 1024], f32r)
            xT = state.tile([128, 512], f32r)   # x^T per batch block [d, n]
            xND = state.tile([128, 512], f32r)  # x per batch block [n, d]
            pall = state.tile([1, 512], f32r)   # prob rows (partition 0)
            w2r = state.tile([128, 256], f32r)  # [W | W] for padded matmuls
            onesr = state.tile([128, 2], f32r)
            ident = state.tile([128, 128], f32)

            bigv = big[:].rearrange("p (u b c) -> p u b c", u=2, b=4)
            adjv = bigv[:, 0]  # [128, 4, 128] adj blocks
            pallv = pall[:].rearrange("p (b c) -> p b c", b=4)

            # ---------------- init ----------------
            x0 = state.tile([128, 512], f32)
            nc.sync.dma_start(
                x0[:].rearrange("n (b d) -> n b d", b=4),
                x_in[:, :, :].rearrange("b n d -> n b d"),
            )
            wtmp = state.tile([128, 128], f32)
            nc.sync.dma_start(wtmp[:], w_in[:, :])
            nc.scalar.copy(
                w2r[:].rearrange("p (u c) -> p u c", u=2),
                wtmp[:].unsqueeze(1).to_broadcast([128, 2, 128]),
            )
            make_identity(nc, ident[:])
            # adj = adjout = I in every block
            nc.scalar.copy(
                big[:].rearrange("p (q c) -> p q c", q=8),
                ident[:].unsqueeze(1).to_broadcast([128, 8, 128]),
            )
            ones_f = state.tile([128, 2], f32)
            nc.vector.memset(ones_f[:], 1.0)
            nc.scalar.copy(onesr[:], ones_f[:])
            pz = state.tile([1, 512], f32)
            nc.vector.memset(pz[:], 0.0)
            nc.scalar.copy(pall[:], pz[:])

            # x0^T via PE transpose, then first conv x1 = relu(x0 @ W)
            x0T_ps = ps_c.tile([128, 512], f32, tag="c")
            for b in range(B_LOC):
                nc.tensor.transpose(
                    x0T_ps[:, b * 128 : (b + 1) * 128],
                    x0[:, b * 128 : (b + 1) * 128],
                    ident[:],
                )
            x0T = work.tile([128, 512], f32r, tag="yTs")
            nc.scalar.copy(x0T[:], x0T_ps[:])
            zT_ps0 = ps_b.tile([128, 512], f32, tag="b")
            nc.tensor.matmul(zT_ps0[:], w2r[:, 0:128], x0T[:], start=True, stop=True)
            nc.scalar.activation(xT[:], zT_ps0[:], AFT.Relu)
            xnd_ps0 = ps_c.tile([128, 512], f32, tag="c")
            for b in range(B_LOC):
                if b < 3:
                    nc.tensor.matmul(
                        xnd_ps0[:, b * 128 : b * 128 + 256],
                        x0T[:, b * 128 : (b + 1) * 128],
                        w2r[:],
                        start=True, stop=True,
                    )
                else:
                    nc.tensor.matmul(
                        xnd_ps0[:, 384:512],
                        x0T[:, 384:512],
                        w2r[:, 0:128],
                        start=True, stop=True,
                    )
            nc.scalar.activation(xND[:], xnd_ps0[:], AFT.Relu)

            # ---------------- the 127 steps ----------------
            for i in range(1, N):
                m = i + 1
                me = m + (m % 2)           # even block size
                mdeg = max(me, 64)         # deg matmul FD padding
                ie = i + (i % 2)           # even prob length
                pc_off = 0 if i < N - 1 else 1

                # 2. prob cols: [i, 2] per batch (col pc_off is the real one)
                pcol = ps_a.tile([128, 512], f32, tag="a")
                for b in range(B_LOC):
                    nc.tensor.matmul(
                        pcol[0:i, 2 * b : 2 * b + 2],
                        xT[:, b * 128 : b * 128 + i],
                        xT[:, b * 128 + i - pc_off : b * 128 + i - pc_off + 2],
                        start=True, stop=True,
                    )
                # 1. prob rows: [1, ie] per batch (partition 0 of psum)
                prow = ps_b.tile([128, 512], f32, tag="b")
                for b in range(B_LOC):
                    nc.tensor.matmul(
                        prow[0:1, b * 128 : b * 128 + ie],
                        xT[:, b * 128 + i : b * 128 + i + 1],
                        xT[:, b * 128 : b * 128 + ie],
                        start=True, stop=True,
                    )
                # 3. prob rows to SBUF (partition 0), f32r; cols >= i stay 0
                nc.scalar.copy(pallv[:, :, 0:i], prow[0:1, :].rearrange(
                    "p (b c) -> p b c", b=4)[:, :, 0:i])
                # 4. column write into adj and adjout (rows 0:i at col i)
                nc.vector.tensor_copy(
                    bigv[0:i, :, :, i : i + 1],
                    pcol[0:i, 0:8].rearrange("p (b t) -> p b t", t=2)
                    [:, :, pc_off : pc_off + 1].unsqueeze(1)
                    .to_broadcast([i, 2, 4, 1]),
                )
                # 5. deg from the stale block (row i not yet written) ...
                deg = ps_b.tile([128, 512], f32, tag="b")
                degv = deg[0:1, 0 : 4 * mdeg].rearrange("p (b c) -> p b c", b=4)
                nc.tensor.matmul(
                    degv[:, :, :],
                    onesr[0:mdeg, 0:1],
                    adjv[0:mdeg, :, 0:mdeg],
                    start=True, stop=False,
                )
                # ... plus the missing row-i contribution (rank-1 fixup)
                nc.tensor.matmul(
                    degv[:, :, :],
                    onesr[0:1, 0:1],
                    pallv[:, :, 0:mdeg],
                    start=False, stop=True,
                    skip_group_check=True,
                )
                # 6. row write via DMA (engines cannot write partition i);
                #    ordered after the deg read, overlaps recip/sqrt/outer.
                nc.sync.dma_start(bigv[i : i + 1, 0, :, 0:i], pallv[:, :, 0:i])
                nc.sync.dma_start(bigv[i : i + 1, 1, :, 0:i], pallv[:, :, 0:i])
                # 7-8. dinv = 1/sqrt(deg)
                rinv = work.tile([1, 512], f32, tag="rinv")
                rinvv = rinv[:].rearrange("p (b c) -> p b c", b=4)
                nc.vector.reciprocal(rinvv[:, :, 0:me], degv[:, :, 0:me])
                dinv = work.tile([1, 512], f32, tag="dinv")
                dinvv = dinv[:].rearrange("p (b c) -> p b c", b=4)
                nc.scalar.sqrt(dinvv[:, :, 0:me], rinvv[:, :, 0:me])
                # 9. S = dinv x dinv (fp32 outer products)
                S = ps_a.tile([128, 512], f32, tag="a")
                for b in range(B_LOC):
                    nc.tensor.matmul(
                        S[0:me, b * 128 : b * 128 + me],
                        dinv[0:1, b * 128 : b * 128 + me],
                        dinv[0:1, b * 128 : b * 128 + me],
                        start=True, stop=True,
                    )
                # 10. adj = adj * S on the me x me blocks (waits for the DMA)
                Sv = S[:].rearrange("p (b c) -> p b c", b=4)
                nc.vector.tensor_mul(
                    adjv[0:me, :, 0:me], adjv[0:me, :, 0:me], Sv[0:me, :, 0:me]
                )
                # 11. y^T = x^T @ adj on the block
                yT = ps_c.tile([128, 512], f32, tag="c")
                for b in range(B_LOC):
                    nc.tensor.matmul(
                        yT[:, b * 128 : b * 128 + me],
                        xND[0:me, b * 128 : (b + 1) * 128],
                        adjv[0:me, b, 0:me],
                        start=True, stop=True,
                    )
                # 12-13. yTs = [yT block | xT passthrough]
                yTs = work.tile([128, 512], f32r, tag="yTs")
                yTv = yT[:].rearrange("p (b c) -> p b c", b=4)
                yTsv = yTs[:].rearrange("p (b c) -> p b c", b=4)
                xTv = xT[:].rearrange("p (b c) -> p b c", b=4)
                nc.vector.tensor_copy(yTsv[:, :, 0:me], yTv[:, :, 0:me])
                if me < 128:
                    nc.gpsimd.tensor_copy(yTsv[:, :, me:128], xTv[:, :, me:128])
                # 14. x_new^T = relu(W^T @ y^T)  (all batches, FD=512)
                zT = ps_b.tile([128, 512], f32, tag="b")
                nc.tensor.matmul(zT[:], w2r[:, 0:128], yTs[:], start=True, stop=True)
                nc.scalar.activation(xT[:], zT[:], AFT.Relu)
                # 15. x_new = relu(y @ W), FD padded to 256 via [W|W]
                xnd_ps = ps_c.tile([128, 512], f32, tag="c")
                for b in range(B_LOC):
                    if b < 3:
                        nc.tensor.matmul(
                            xnd_ps[:, b * 128 : b * 128 + 256],
                            yTs[:, b * 128 : (b + 1) * 128],
                            w2r[:],
                            start=True, stop=True,
                        )
                    else:
                        nc.tensor.matmul(
                            xnd_ps[:, 384:512],
                            yTs[:, 384:512],
                            w2r[:, 0:128],
                            start=True, stop=True,
                        )
                nc.scalar.activation(xND[:], xnd_ps[:], AFT.Relu)

            # ---------------- write out adj_output ----------------
            stage = state.tile([128, 512], f32)
            nc.scalar.copy(stage[:], big[:, 512:1024])
            nc.sync.dma_start(
                out_d[:, :, :].rearrange("b n c -> n b c"),
                stage[:].rearrange("n (b c) -> n b c", b=4),
            )

    nc.finalize()
    return nc


def _get_nc():
    if "nc" not in _nc_cache:
        _nc_cache["nc"] = _build_nc()
    return _nc_cache["nc"]


def kernel(**inputs):
    import sys

    if "/opt/trn_rl_repo" not in sys.path:
        sys.path.insert(0, "/opt/trn_rl_repo")
    from concourse.bass_utils import run_bass_kernel_spmd

    x = np.ascontiguousarray(np.asarray(inputs["x"], dtype=np.float32))
    W = np.ascontiguousarray(np.asarray(inputs["W"], dtype=np.float32))
    nc = _get_nc()
    in_maps = [
        {"x": x[c * B_LOC : (c + 1) * B_LOC], "W": W} for c in range(N_CORES)
    ]
    res = run_bass_kernel_spmd(nc, in_maps, core_ids=list(range(N_CORES)))
    return np.concatenate([r["out"] for r in res.results], axis=0)
